# revision 37
# baseline (speedup 1.0000x reference)
"""Trainium2 Bass kernel for the exponential-kernel multivariate Hawkes
process log-likelihood (B=4, N=2048, D=32).

Strategy
--------
The log-likelihood per batch is
  pos  = sum_i log( mu[d_i] + sum_{j<i} a[d_i,d_j] b[d_i,d_j] e^{-b(t_i-t_j)} )
  neg  = -sum_d ( mu_d T + sum_j a[d,d_j] (1 - e^{-b[d,d_j](T-t_j)}) )

Each pairwise term is one exponential:
  a b e^{-b (t_i - t_j)} = exp( b[d_i,d_j] t_j + (ln(ab)[d_i,d_j] - b[d_i,d_j] t_i) )
Both exponent terms are bilinear in one-hot encodings of the event types, so a
[128 rows x W cols] tile of exponents z is a small-K matmul of per-row tables
  beta_rowsT[k,i] = b[d_i,k],   l23[k,i] = ln(ab)[d_i,k] - t_i b[d_i,k]
against one-hot column streams.  The matmuls run in bf16 with an exact hi/lo
splitting (fp32 streams 4x slower per column through the PE):
  b t_j = b_hi t_hi + b_hi t_lo + b_lo t_hi (+ dropped b_lo t_lo ~ 2e-3)
  l23   = l23_hi + l23_lo
where *_hi = bf16 round, *_lo = bf16(residual); b_hi*t_hi products are exact
in bf16 thanks to the one-hot structure.  Four of the five terms form a single
K=128 bf16 matmul ([b_hi; b_hi; l23_hi; l23_lo] x [ETs_hi; ETs_lo; ET; ET]),
the fifth (b_lo x ETs_hi) is a K=32 matmul into the same PSUM accumulation.
ScalarE Exp with accum_out yields the row-sums sum_j directly; per-row-tile
intensities add the mu gather (a K=64 exact-bf16 matmul) and go through Ln and
tree reductions on-device.  The compensator uses the same exponent-matmul over
the event list.  All O(N*D) table/one-hot encoding is host-side input prep;
the O(N^2) pairwise work, exp/log, and reductions run on the NeuronCores.

Sharding: 8 cores = 4 batches x 2 halves.  All cores run ONE identical
program (SPMD); which batch / row-tiles / column ranges a core computes is
decided entirely by host-arranged input streams.  Row-tiles of the
lower-triangular [N,N] interaction are dealt so both halves get identical
piece-count profiles; strips are padded to fixed widths with sentinel columns
(ETs_hi = -1e4 * e_0) whose exponent is < -1000 so they contribute exactly 0.
The diagonal 128-block at the end of every strip is masked in PSUM with an
additive -30000 strict-lower-triangular tile before the Exp.
"""

import numpy as np
import ml_dtypes
from contextlib import ExitStack

import concourse.bass as bass
import concourse.bacc as bacc
import concourse.mybir as mybir
import concourse.tile as tile
from concourse.bass_utils import run_bass_kernel_spmd

F32 = mybir.dt.float32
BF16 = mybir.dt.bfloat16
AF = mybir.ActivationFunctionType
BF16NP = np.dtype(ml_dtypes.bfloat16)

B, N, D = 4, 2048, 32

# Row-tile deal between the two cores of a batch: identical piece profiles.
TILES = ((0, 3, 4, 7, 8, 11, 12, 15), (1, 2, 5, 6, 9, 10, 13, 14))
NPIECES = (1, 1, 1, 1, 2, 2, 2, 2)          # 1024-wide pieces per strip slot
WLAST = (256, 512, 768, 1024, 256, 512, 768, 1024)  # width of last piece
SLOT_TOT = tuple((n - 1) * 1024 + w for n, w in zip(NPIECES, WLAST))
SSTREAM = sum(SLOT_TOT)  # 9216 columns streamed per core
PAD_SENTINEL = -1.0e4    # ETs_hi value for padding columns
MASK_NEG = -30000.0      # additive mask for diagonal-tile upper half

_PROGRAM = None


def _build_program():
    nc = bacc.Bacc("TRN2", target_bir_lowering=False, debug=False, num_devices=8)

    def din(name, shape, dt=BF16):
        return nc.dram_tensor(name, shape, dt, kind="ExternalInput").ap()

    # cols_cat: 0-31 ET, 32-63 ETs_hi, 64-95 ETs_lo
    cols_cat = din("cols_cat", [96, SSTREAM])
    # rows_cat: 0-31 ET, 32-63 ETs_hi, 64-95 ETs_lo
    rows_cat = din("rows_cat", [96, 1024])
    lhsT96 = din("lhsT96", [96, 1024])   # [l23_hi; b_hi; b_hi]
    lhsT64 = din("lhsT64", [64, 1024])   # [l23_lo; b_lo]
    negL96 = din("negL96", [96, D])      # [g_hi; bT_hi; bT_hi]
    negL64 = din("negL64", [64, D])      # [g_lo; bT_lo]
    muLhi = din("muLhi", [D, 1])
    muLlo = din("muLlo", [D, 1])
    alphaT = din("alphaT", [D, D], F32)
    muf = din("muf", [D, 1], F32)
    mut = din("mut", [D, 1], F32)
    cnt = din("cnt", [D, 1], F32)
    mask = din("mask", [128, 128], F32)
    out = nc.dram_tensor("out", [1, 1], F32, kind="ExternalOutput").ap()

    with tile.TileContext(nc) as tc:
        with ExitStack() as ctx:
            _emit(ctx, tc, nc, cols_cat, rows_cat, lhsT96, lhsT64,
                  negL96, negL64, muLhi, muLlo, alphaT, muf, mut, cnt,
                  mask, out)
    nc.compile()
    return nc


def _emit(ctx, tc, nc, cols_cat, rows_cat, lhsT96_d, lhsT64_d,
          negL96_d, negL64_d, muLhi_d, muLlo_d, alphaT_d, muf_d, mut_d,
          cnt_d, mask_d, out):
    const = ctx.enter_context(tc.tile_pool(name="const", bufs=1))
    scratch = ctx.enter_context(tc.tile_pool(name="scratch", bufs=3))
    small = ctx.enter_context(tc.tile_pool(name="small", bufs=2))
    accp = ctx.enter_context(tc.tile_pool(name="accp", bufs=4))
    psum_z = ctx.enter_context(tc.tile_pool(name="psum_z", bufs=4, space="PSUM"))

    # Preload the Exp activation table while DMAs are in flight (dep-free).
    d0 = small.tile([D, 1], F32, tag="d0")
    nc.vector.memset(d0[:], 0.0)
    dexp = small.tile([D, 1], F32, tag="dexp")
    nc.scalar.activation(dexp[:], d0[:], AF.Exp)

    # ---- loads: latency-critical tables first on HWDGE, then the stream;
    # non-critical tables on SWDGE (gpsimd) in parallel.
    def sload(ap, shape, tag, dt=BF16):
        t = const.tile(shape, dt, tag=tag)
        nc.sync.dma_start(t[:], ap)
        return t

    def gload(ap, shape, tag, dt=BF16):
        t = const.tile(shape, dt, tag=tag)
        nc.gpsimd.dma_start(t[:], ap)
        return t

    lhsT96 = sload(lhsT96_d, [96, 1024], "lhsT96")
    lhsT64 = sload(lhsT64_d, [64, 1024], "lhsT64")
    cols_t = const.tile([96, SSTREAM], BF16, tag="cols")
    c0 = 0
    for cw in (512, 1024, 1536, 1536, 1536, 1536, 1536):
        nc.sync.dma_start(cols_t[:, c0 : c0 + cw], cols_cat[:, c0 : c0 + cw])
        c0 += cw

    rows_t = gload(rows_cat, [96, 1024], "rows")
    mask_t = gload(mask_d, [128, 128], "mask", F32)
    negL96 = gload(negL96_d, [96, D], "negL96")
    negL64 = gload(negL64_d, [64, D], "negL64")
    muLhi = gload(muLhi_d, [D, 1], "muLhi")
    muLlo = gload(muLlo_d, [D, 1], "muLlo")
    alphaT_t = gload(alphaT_d, [D, D], "alphaT", F32)
    muf_t = gload(muf_d, [D, 1], "muf", F32)
    mut_t = gload(mut_d, [D, 1], "mut", F32)
    cnt_t = gload(cnt_d, [D, 1], "cnt", F32)



    mu_cols = const.tile([128, 8], F32, tag="mu_cols")
    lam_cols = const.tile([128, 8], F32, tag="lam_cols")

    # ---- main loop: 8 strip slots, fixed piece structure ----------------
    off = 0
    for s in range(8):
        npc = NPIECES[s]
        rsl = slice(s * 128, (s + 1) * 128)
        acc = accp.tile([128, 2], F32, tag="acc")
        for p in range(npc):
            w = 1024 if p < npc - 1 else WLAST[s]
            z = psum_z.tile([128, 1024], F32, tag="z")
            # same-weight matmuls grouped to halve PE weight reloads
            for g0 in range(0, w, 512):
                gw = min(512, w - g0)
                csl = slice(off + g0, off + g0 + gw)
                nc.tensor.matmul(z[:, g0 : g0 + gw], lhsT96[:, rsl],
                                 cols_t[:, csl], start=True, stop=False)
            for g0 in range(0, w, 512):
                gw = min(512, w - g0)
                csl = slice(off + g0, off + g0 + gw)
                nc.tensor.matmul(z[:, g0 : g0 + gw], lhsT64[:, rsl],
                                 cols_t[0:64, csl], start=False, stop=True)
            if p == npc - 1:
                # mask the diagonal 128-block (last 128 cols) in place
                nc.vector.tensor_add(z[:, w - 128 : w], z[:, w - 128 : w],
                                     mask_t[:])
            e1 = scratch.tile([128, 1024], BF16, tag="e1")
            nc.scalar.activation(e1[:, :w], z[:, :w], AF.Exp,
                                 accum_out=acc[:, p : p + 1])
            off += w

        nc.vector.reduce_sum(lam_cols[:, s : s + 1], acc[:, :npc],
                             axis=mybir.AxisListType.X)

    # mu_cols[i, s] = mu[d_i] per row-tile slot (K=32 bf16 hi+lo, exact)
    mu_ps = psum_z.tile([128, 8], F32, tag="z")
    for s in range(8):
        ssl = slice(s * 128, (s + 1) * 128)
        nc.tensor.matmul(mu_ps[:, s : s + 1], rows_t[0:D, ssl], muLhi[:],
                         start=True, stop=False)
        nc.tensor.matmul(mu_ps[:, s : s + 1], rows_t[0:D, ssl], muLlo[:],
                         start=False, stop=True)
    nc.vector.tensor_copy(mu_cols[:], mu_ps[:])

    # compensator over the core's 1024 events
    z2 = psum_z.tile([D, 1024], F32, tag="z")
    for q in range(2):
        sl = slice(q * 512, q * 512 + 512)
        nc.tensor.matmul(z2[:, sl], negL96[:], rows_t[:, sl],
                         start=True, stop=False)
        nc.tensor.matmul(z2[:, sl], negL64[:], rows_t[0:64, sl],
                         start=False, stop=True)
    negexp_sum = small.tile([D, 1], F32, tag="nes")
    e2n = scratch.tile([D, 1024], BF16, tag="e2n")
    nc.scalar.activation(e2n[:], z2[:], AF.Exp, accum_out=negexp_sum[:])

    # ---- final reduction ------------------------------------------------
    lam2 = const.tile([128, 8], F32, tag="lam2")
    nc.vector.tensor_add(lam2[:], lam_cols[:], mu_cols[:])
    loglam = const.tile([128, 8], F32, tag="loglam")
    nc.scalar.activation(loglam[:], lam2[:], AF.Ln)

    pos_vec = small.tile([128, 1], F32, tag="posv")
    nc.vector.reduce_sum(pos_vec[:], loglam[:], axis=mybir.AxisListType.X)

    acs = psum_z.tile([D, 1], F32, tag="z")
    nc.tensor.matmul(acs[:], alphaT_t[:], cnt_t[:], start=True, stop=True)
    v = small.tile([D, 1], F32, tag="v")
    nc.vector.tensor_sub(v[:], acs[:], negexp_sum[:])  # sum_j alpha - sum_j e2
    muTv = small.tile([D, 1], F32, tag="mutv")
    nc.vector.tensor_mul(muTv[:], muf_t[:], mut_t[:])
    v2 = small.tile([D, 1], F32, tag="v2")
    nc.vector.tensor_add(v2[:], v[:], muTv[:])

    ones128 = const.tile([128, 1], F32, tag="ones128")
    nc.vector.memset(ones128[:], 1.0)
    ones32 = const.tile([D, 1], F32, tag="ones32")
    nc.vector.memset(ones32[:], 1.0)

    tpos = psum_z.tile([1, 1], F32, tag="z")
    nc.tensor.matmul(tpos[:], ones128[:], pos_vec[:], start=True, stop=True)
    tneg = psum_z.tile([1, 1], F32, tag="z")
    nc.tensor.matmul(tneg[:], ones32[:], v2[:], start=True, stop=True)
    tpos_sb = small.tile([1, 1], F32, tag="tpossb")
    nc.vector.tensor_copy(tpos_sb[:], tpos[:])
    res = small.tile([1, 1], F32, tag="res")
    nc.vector.tensor_sub(res[:], tpos_sb[:], tneg[:])
    nc.sync.dma_start(out, res[:])


def _bf(x):
    return x.astype(BF16NP)


def _split(x):
    hi = _bf(x)
    lo = _bf(x - hi.astype(np.float32))
    return hi, lo


def _host_prep(time_points, T, mu_raw, alpha_raw, beta_raw, event_types):
    time_points = np.ascontiguousarray(np.asarray(time_points, dtype=np.float32))
    T = np.asarray(T, dtype=np.float32)
    mu_raw = np.asarray(mu_raw, dtype=np.float32).reshape(D)
    alpha_raw = np.asarray(alpha_raw, dtype=np.float32)
    beta_raw = np.asarray(beta_raw, dtype=np.float32)
    event_types = np.asarray(event_types).astype(np.int64)

    def softplus(x):
        return np.log1p(np.exp(x)).astype(np.float32)

    mu = softplus(mu_raw)          # (D,)
    alpha = softplus(alpha_raw)    # (D,D) receiver x trigger
    beta = softplus(beta_raw)
    lnab = np.log(alpha * beta).astype(np.float32)
    lnalpha = np.log(alpha).astype(np.float32)
    mu_hi, mu_lo = _split(mu.reshape(D, 1))

    # strict-lower keep mask for the diagonal 128-block (0 keep / MASK_NEG drop)
    ii = np.arange(128)
    mask = np.where(ii[None, :] < ii[:, None], 0.0, MASK_NEG).astype(np.float32)

    in_maps = []
    for c in range(8):
        b, h = c // 2, c % 2
        tp = time_points[b]
        et = event_types[b]
        t_hi = tp.astype(BF16NP).astype(np.float32)
        t_lo = tp - t_hi
        onehotT = np.zeros((D, N), dtype=np.float32)
        onehotT[et, np.arange(N)] = 1.0

        g_list = TILES[h]
        rows_idx = np.concatenate(
            [np.arange(g * 128, (g + 1) * 128) for g in g_list])
        et_r = et[rows_idx]
        t_r = tp[rows_idx]
        oh_rows = onehotT[:, rows_idx]
        rows_cat = np.zeros((96, 1024), dtype=BF16NP)
        rows_cat[0:D] = _bf(oh_rows)
        rows_cat[D : 2 * D] = _bf(oh_rows * t_hi[rows_idx][None, :])
        rows_cat[2 * D : 3 * D] = _bf(oh_rows * t_lo[rows_idx][None, :])

        # per-row tables: beta_rows[k,i] = beta[d_i,k], l23 = lnab - t_i*beta
        beta_rows = beta[et_r, :].T.astype(np.float32)          # [D, 1024]
        l23 = (lnab[et_r, :].T - t_r[None, :] * beta_rows).astype(np.float32)
        bh, bl = _split(beta_rows)
        lh, ll = _split(l23)
        lhsT96 = np.concatenate([lh, bh, bh], axis=0)   # [96, 1024]
        lhsT64 = np.concatenate([ll, bl], axis=0)       # [64, 1024]

        # compensator tables: z2 = lnalpha[d,dj] - beta[d,dj]*(T - t_j)
        g = (lnalpha.T - T[b] * beta.T).astype(np.float32)      # [k, d]
        gh, gl = _split(g)
        bTh, bTl = _split(beta.T.astype(np.float32))
        negL96 = np.concatenate([gh, bTh, bTh], axis=0)  # [96, D]
        negL64 = np.concatenate([gl, bTl], axis=0)       # [64, D]

        cols_cat = np.zeros((96, SSTREAM), dtype=BF16NP)
        off = 0
        for s, gidx in enumerate(g_list):
            tot = SLOT_TOT[s]
            real = (gidx + 1) * 128
            pad = tot - real
            cols_cat[D, off : off + pad] = PAD_SENTINEL  # ETs_hi row k=0
            r = slice(off + pad, off + tot)
            cols_cat[0:D, r] = _bf(onehotT[:, :real])
            cols_cat[D : 2 * D, r] = _bf(onehotT[:, :real] * t_hi[None, :real])
            cols_cat[2 * D : 3 * D, r] = _bf(onehotT[:, :real] * t_lo[None, :real])
            off += tot

        cntv = np.bincount(et_r, minlength=D).astype(np.float32).reshape(D, 1)
        mutv = np.full((D, 1), T[b] if h == 0 else 0.0, dtype=np.float32)

        in_maps.append(dict(
            cols_cat=cols_cat, rows_cat=rows_cat,
            lhsT96=lhsT96, lhsT64=lhsT64, negL96=negL96, negL64=negL64,
            muLhi=mu_hi, muLlo=mu_lo,
            alphaT=np.ascontiguousarray(alpha.T).astype(np.float32),
            muf=mu.reshape(D, 1).astype(np.float32),
            mut=mutv, cnt=cntv, mask=mask,
        ))
    return in_maps


_LAST_RESULTS = None  # BassKernelResults of the most recent run (for test.py)


def kernel(time_points, T, mu_raw, alpha_raw, beta_raw, event_types,
           _trace=False):
    global _PROGRAM, _LAST_RESULTS
    if _PROGRAM is None:
        _PROGRAM = _build_program()
    nc = _PROGRAM
    in_maps = _host_prep(time_points, T, mu_raw, alpha_raw, beta_raw, event_types)
    res = run_bass_kernel_spmd(nc, in_maps, list(range(8)), trace=_trace)
    _LAST_RESULTS = res
    partial = np.array(
        [np.asarray(res.results[c]["out"]).reshape(()) for c in range(8)],
        dtype=np.float32)
    return (partial[0::2] + partial[1::2]).astype(np.float32)
